# revision 67
# baseline (speedup 1.0000x reference)
"""Transformer block (LN -> MHA -> residual -> LN -> MLP -> residual) on 8 TRN2
NeuronCores.

Sharding: pure row data-parallelism over (batch, sequence-half). Core c handles
batch b = c//2 and query rows [h*512, (h+1)*512) with h = c%2. Each core
computes K/V projections for its full batch locally (small duplicated work),
which removes every cross-core collective. Host reorders each core's batch rows
"own rows first" so the same SPMD program works on all cores; mask columns are
permuted identically (softmax/attention are permutation-invariant over keys).

Optimizations vs the bf16 baseline (1142us -> 819us):
  - QKV + output projections run in fp8 (e4m3) DoubleRow mode (2 k-tiles
    contracted per matmul, ~1.7x PE throughput): weights are pre-scaled x512
    on the host (clears e4m3's subnormal zone) and the 1/512 descale folds
    into the existing bias ops. MLP stays bf16 (fp8 there exceeds the 2e-2
    error budget; measured in numpy simulation).
  - Both LayerNorms' affine (w, b) fold into the downstream projection
    weights/biases on the host, so the device only computes (x-mu)*rstd;
    the pre-MLP LN output feeds MLP1 directly with no apply pass.
  - LN1 runs lo-half first so the four heads' Q projections start the PE
    ~60us in; the hi half (for K/V) normalizes underneath the Q matmuls.
  - scores(h+1) is emitted before av(h) so the last head's softmax hides
    behind av matmuls instead of stalling the PE.
  - The output projection preloads wo during attention (own pool, no WAR on
    attention tiles), runs qt-outer with fc-inner (4 matmuls per weight
    load), and LN2 pipelines per row-tile; MLP1's first fo-tiles run as
    row-halves so the PE bridges the LN2 tail.
  - MLP2 streams w2 in half-column panels with hT stationary (2 matmuls per
    weight load); the last 8 hidden tiles finish per-qt so drains stagger.
  - DMA issues are spread across the sync/scalar/gpsimd sequencers (~0.6us
    per issue each) to avoid head-of-line blocking on one queue.
Matmuls accumulate in fp32 PSUM; statistics stay fp32; residual r is bf16
via a DRAM bounce that stays off the critical path.
"""

import numpy as np
import ml_dtypes

import concourse.bass as bass
import concourse.tile as tile
from concourse import bacc, mybir
from concourse.bass_utils import run_bass_kernel_spmd

BF16 = mybir.dt.bfloat16
F32 = mybir.dt.float32
FP8 = mybir.dt.float8e4
AX = mybir.AxisListType
OP = mybir.AluOpType
ACT = mybir.ActivationFunctionType
DR = mybir.MatmulPerfMode.DoubleRow

P = 128
B, T, C, H = 4, 1024, 2048, 4
DH = C // H                      # 512
F = 4 * C                        # 8192
R = T // 2                       # 512 own query rows per core
RT, TT, CT, FT = R // P, T // P, C // P, F // P   # 4, 8, 16, 64
HT = DH // P                     # 4 feature tiles per head
KP = CT // 2                     # 8 k-pair (DoubleRow) steps over C
EPS = 1e-5
ISQ = 1.0 / float(np.sqrt(DH))
NEGBIG = 30000.0
WS = 512.0                       # fp8 weight pre-scale
IWS = 1.0 / WS


def _bcast_load(nc, pool, dram_ap, name, dtype):
    """Broadcast a [n] DRAM vector to all 128 partitions -> [128, n]."""
    t = pool.tile([P, dram_ap.shape[0]], dtype, name=name)
    src = bass.AP(
        tensor=dram_ap.tensor, offset=dram_ap.offset, ap=[[0, P]] + list(dram_ap.ap)
    )
    nc.gpsimd.dma_start(out=t[:], in_=src)
    return t


def _ln_stats_xh(nc, pool, xs, eps_t, tag, sink):
    """LayerNorm stats + normalize of f32 [128, C] tiles, phase-batched.
    Produces xh = (x-mu)*rstd in bf16 via sink(i, xh_tile); the w*xh+b apply
    happens later in transposed space (fused into the ACT slab cast)."""
    n = len(xs)
    mvs, nmrs, rstds = [], [], []
    for i, x_sl in enumerate(xs):
        stats = pool.tile([P, 4, 6], F32, name=f"{tag}_stats{i}", tag=f"{tag}_stats", bufs=n)
        for sg in range(4):
            nc.vector.bn_stats(out=stats[:, sg, :], in_=x_sl[:, sg * 512:(sg + 1) * 512])
        mv = pool.tile([P, 2], F32, name=f"{tag}_mv{i}", tag=f"{tag}_mv", bufs=n)
        nc.vector.bn_aggr(out=mv[:], in_=stats[:])
        mvs.append(mv)
    stds = []
    for i in range(n):
        std = pool.tile([P, 1], F32, name=f"{tag}_std{i}", tag=f"{tag}_std", bufs=n)
        nc.scalar.activation(out=std[:], in_=mvs[i][:, 1:2], func=ACT.Sqrt,
                             bias=eps_t[:], scale=1.0)
        stds.append(std)
    for i in range(n):
        rstd = pool.tile([P, 1], F32, name=f"{tag}_rstd{i}", tag=f"{tag}_rstd", bufs=n)
        nc.vector.reciprocal(rstd[:], stds[i][:])
        nmr = pool.tile([P, 1], F32, name=f"{tag}_nmr{i}", tag=f"{tag}_nmr", bufs=n)
        nc.vector.tensor_scalar(nmr[:], mvs[i][:, 0:1], rstd[:], -1.0, OP.mult, OP.mult)
        rstds.append(rstd)
        nmrs.append(nmr)
    for i in range(n):
        xh = pool.tile([P, C], BF16, name=f"{tag}_xh{i}", tag=f"{tag}_xh", bufs=2)
        nc.scalar.activation(out=xh[:], in_=xs[i], func=ACT.Identity,
                             bias=nmrs[i][:], scale=rstds[i][:])
        sink(i, xh)


def _body(tc):
    nc = tc.nc
    d = {n: nc.dram_tensor(n, s, dt, kind=k).ap() for n, s, dt, k in [
        ("x", [T, C], F32, "ExternalInput"),
        ("mask", [R, T], BF16, "ExternalInput"),
        ("wq", [CT, P, CT, P], FP8, "ExternalInput"),
        ("wk", [CT, P, CT, P], FP8, "ExternalInput"),
        ("wv", [H, P, CT, DH], FP8, "ExternalInput"),
        ("wo", [KP, P, 2, 4, 512], FP8, "ExternalInput"),
        ("w1", [FT // 2, P, CT, 2, P], BF16, "ExternalInput"),
        ("w2", [2, FT, P, 1024], BF16, "ExternalInput"),
        ("bq", [P, CT], F32, "ExternalInput"),
        ("bk", [P, CT], F32, "ExternalInput"),
        ("b1", [P, FT], F32, "ExternalInput"),
        ("bv", [C], BF16, "ExternalInput"),
        ("bo", [C], BF16, "ExternalInput"),
        ("b2", [C], BF16, "ExternalInput"),

        ("out", [R, C], F32, "ExternalOutput"),
    ]}

    consts = tc.alloc_tile_pool(name="consts", bufs=1)
    eps_t = consts.tile([P, 1], F32, name="eps")
    nc.vector.memset(eps_t[:], EPS)
    bq_t = consts.tile([P, CT], F32, name="bq_t")
    nc.sync.dma_start(bq_t[:], d["bq"])
    bk_t = consts.tile([P, CT], F32, name="bk_t")
    nc.sync.dma_start(bk_t[:], d["bk"])

    # Pool stack, ordered by lifetime (released LIFO):
    p_rd = tc.alloc_tile_pool(name="p_rd", bufs=1, space="DRAM")
    r_d = p_rd.tile([R, C], BF16, name="r_d")             # residual bounce for stage G
    p_xn2T = tc.alloc_tile_pool(name="p_xn2T", bufs=1)    # dies after G (kept open)
    xh2T = p_xn2T.tile([P, CT, R], BF16, name="xh2T")     # LN2-normalized, transposed
    # (l2w/l2b fold into W1/b1 on the host, so xh2T feeds MLP1 directly)
    p_yT = tc.alloc_tile_pool(name="p_yT", bufs=1)        # dies after D
    yT8 = p_yT.tile([P, CT, R], FP8, name="yT8")
    p_xnT8 = tc.alloc_tile_pool(name="p_xnT8", bufs=1)    # dies after D (used in C)
    xnT8_lo = p_xnT8.tile([P, CT, R], FP8, name="xnT8_lo")
    xnT8_hi = p_xnT8.tile([P, CT, R], FP8, name="xnT8_hi")
    xnT8 = [xnT8_lo, xnT8_hi]
    p_w1h = tc.alloc_tile_pool(name="p_w1h", bufs=1)      # w1 head chunks; dies after F
    NSPLIT = 2
    w1hs = [p_w1h.tile([P, CT, 2, P], BF16, name=f"w1h{i}") for i in range(NSPLIT)]
    p_wo = tc.alloc_tile_pool(name="p_wo", bufs=1)        # preloaded in C, used in D
    wo_t = p_wo.tile([P, KP, 2, 4, 512], FP8, name="wo_t")
    pQ = tc.alloc_tile_pool(name="pQ", bufs=2)            # qTh; dies after C

    # ---------------- Stage A: LN1 (lo then hi) -> transpose -> fused apply+fp8
    pA = tc.alloc_tile_pool(name="pA", bufs=2)

    xts_lo, xts_hi = [], []
    for tt in range(RT):
        # split col-quarters across queues so bn_stats starts early
        xt = pA.tile([P, C], F32, name=f"xlo{tt}", tag="xlo", bufs=RT)
        for sg in range(4):
            cs = slice(sg * 512, (sg + 1) * 512)
            nc.sync.dma_start(xt[:, cs], d["x"][tt * P:(tt + 1) * P, cs])
        xts_lo.append(xt[:])
    for tt in range(RT):
        xt = pA.tile([P, C], F32, name=f"xhi{tt}", tag="xhi", bufs=2)
        for sg in range(2):
            cs = slice(sg * 1024, (sg + 1) * 1024)
            nc.scalar.dma_start(xt[:, cs], d["x"][(RT + tt) * P:(RT + tt + 1) * P, cs])
        xts_hi.append(xt[:])

    def _sink_ln1(tt, xh_t):
        # transpose, then cast the landed column block to fp8 (l1w/l1b are
        # folded into Wq/Wk/Wv host-side, so this is a pure cast)
        half, lt = divmod(tt, RT)
        xhTt = pA.tile([P, CT, P], BF16, name=f"xhT{tt}", tag="xhT", bufs=3)
        nc.sync.dma_start_transpose(xhTt[:], xh_t[:])
        nc.scalar.activation(out=xnT8[half][:, :, lt * P:(lt + 1) * P],
                             in_=xhTt[:], func=ACT.Copy)

    _ln_stats_xh(nc, pA, xts_lo, eps_t, "ln1", lambda i, t_: _sink_ln1(i, t_))
    _ln_stats_xh(nc, pA, xts_hi, eps_t, "ln1", lambda i, t_: _sink_ln1(i + RT, t_))
    pA.release()

    # ---------------- Stage B: Q for ALL heads (needs lo half only)
    psQ = tc.alloc_tile_pool(name="psQ", bufs=8, space="PSUM")
    qThs = {}
    for h in range(H):
        qTh = pQ.tile([P, HT, R], BF16, name=f"qTh{h}", tag="qTh", bufs=H)
        for fl in range(HT):
            fo = h * HT + fl
            wqc = pQ.tile([P, CT, P], FP8, name="wqc", tag="wqc", bufs=4)
            nc.gpsimd.dma_start(wqc[:], d["wq"][fo])
            ps_q = psQ.tile([P, R], F32, name="ps_q", tag="psQ", bufs=8)
            for kp in range(KP):
                nc.tensor.matmul(ps_q[:], wqc[:, 2 * kp:2 * kp + 2, :],
                                 xnT8_lo[:, 2 * kp:2 * kp + 2, :],
                                 start=(kp == 0), stop=(kp == KP - 1), perf_mode=DR)
            nc.vector.tensor_scalar(qTh[:, fl, :], ps_q[:], IWS,
                                    bq_t[:, fo:fo + 1], OP.mult, OP.add)
        qThs[h] = qTh
    psQ.release()
    # preload the output-projection weights while attention runs (own pool, so
    # the DMAs don't alias attention tiles and start immediately)
    for fip in range(KP):
        nc.sync.dma_start(wo_t[:, fip, :, :, :], d["wo"][fip])

    # remaining constants (emitted late so their DMAs don't delay stage A)
    bv_bc = _bcast_load(nc, consts, d["bv"], "bv_bc", BF16)
    bo_bc = _bcast_load(nc, consts, d["bo"], "bo_bc", BF16)
    b2_bc = _bcast_load(nc, consts, d["b2"], "b2_bc", BF16)
    b1_t = consts.tile([P, FT], F32, name="b1_t")
    nc.sync.dma_start(b1_t[:], d["b1"])
    # mask -> additive bias: 0 where visible, -30000 where masked
    mb = consts.tile([P, RT, T], BF16, name="mb")
    nc.sync.dma_start(mb[:], d["mask"].rearrange("(qo qp) k -> qp qo k", qp=P))
    nc.vector.tensor_scalar(mb[:], mb[:], NEGBIG, -NEGBIG, OP.mult, OP.add)

    # ---------------- Stage C: software-pipelined per-head K/V + attention
    pBC = tc.alloc_tile_pool(name="pBC", bufs=2)
    psBC = tc.alloc_tile_pool(name="psBC", bufs=2, space="PSUM")
    hs = {}

    def emit_kv(h):
        kTh = pBC.tile([P, HT, T], BF16, name=f"kTh{h}", tag="kTh", bufs=2)
        for fl in range(HT):
            fo = h * HT + fl
            wkc = pBC.tile([P, CT, P], FP8, name="wkc", tag="wkc", bufs=2)
            nc.gpsimd.dma_start(wkc[:], d["wk"][fo])
            for nn in range(2):
                ps_k = psBC.tile([P, 512], F32, name="ps_k", tag="psB", bufs=2)
                for kp in range(KP):
                    nc.tensor.matmul(ps_k[:], wkc[:, 2 * kp:2 * kp + 2, :],
                                     xnT8[nn][:, 2 * kp:2 * kp + 2, :],
                                     start=(kp == 0), stop=(kp == KP - 1), perf_mode=DR)
                nc.scalar.activation(out=kTh[:, fl, nn * 512:(nn + 1) * 512], in_=ps_k[:],
                                     func=ACT.Identity, bias=bk_t[:, fo:fo + 1], scale=IWS)
        vh = pBC.tile([P, TT, DH], BF16, name=f"vh{h}", tag="vh", bufs=2)
        wvc = pBC.tile([P, CT, DH], FP8, name="wvc", tag="wvc", bufs=1)
        nc.gpsimd.dma_start(wvc[:], d["wv"][h])
        for to in range(TT):
            half, lt = divmod(to, RT)
            ps_v = psBC.tile([P, DH], F32, name="ps_v", tag="psB", bufs=2)
            for kp in range(KP):
                nc.tensor.matmul(ps_v[:], xnT8[half][:, 2 * kp:2 * kp + 2, lt * P:(lt + 1) * P],
                                 wvc[:, 2 * kp:2 * kp + 2, :],
                                 start=(kp == 0), stop=(kp == KP - 1), perf_mode=DR)
            nc.vector.scalar_tensor_tensor(vh[:, to, :], ps_v[:], IWS,
                                           bv_bc[:, h * DH:(h + 1) * DH], OP.mult, OP.add)
        hs[h] = (kTh, vh)

    def emit_scores(h):
        kTh, _ = hs[h]
        qTh = qThs.pop(h)
        attT = pBC.tile([P, TT, R], BF16, name=f"attT{h}", tag="attT", bufs=2)
        for qt in range(RT):
            ps_s = psBC.tile([P, T], F32, name="ps_s", tag="scores", bufs=2)
            for nn in range(2):
                for di in range(HT):
                    nc.tensor.matmul(
                        ps_s[:, nn * 512:(nn + 1) * 512],
                        qTh[:, di, qt * P:(qt + 1) * P],
                        kTh[:, di, nn * 512:(nn + 1) * 512],
                        start=(di == 0), stop=(di == HT - 1))
            s_sb = pBC.tile([P, T], F32, name="s_sb", tag="s_sb", bufs=2)
            nc.vector.scalar_tensor_tensor(s_sb[:], ps_s[:], ISQ, mb[:, qt, :],
                                           OP.mult, OP.add)
            negmax = pBC.tile([P, 1], F32, name="negmax", tag="negmax", bufs=2)
            nc.vector.reduce_max(negmax[:], s_sb[:], axis=AX.X, negate=True)
            e_sb = pBC.tile([P, T], BF16, name="e_sb", tag="e_sb", bufs=2)
            sums = pBC.tile([P, 1], F32, name="sums", tag="sums", bufs=2)
            nc.scalar.activation(out=e_sb[:], in_=s_sb[:], func=ACT.Exp,
                                 bias=negmax[:], scale=1.0, accum_out=sums[:])
            recip = pBC.tile([P, 1], F32, name="recip", tag="recip", bufs=2)
            nc.vector.reciprocal(recip[:], sums[:])
            nc.vector.tensor_scalar_mul(e_sb[:], e_sb[:], recip[:])
            nc.scalar.dma_start_transpose(attT[:, :, qt * P:(qt + 1) * P], e_sb[:])
        hs[h] = hs[h] + (attT,)

    def emit_av(h):
        _, vh, attT = hs.pop(h)
        for dt_ in range(HT):
            ps_y = psBC.tile([P, R], F32, name="ps_y", tag="av", bufs=2)
            for ko in range(TT):
                nc.tensor.matmul(ps_y[:], vh[:, ko, dt_ * P:(dt_ + 1) * P],
                                 attT[:, ko, :], start=(ko == 0), stop=(ko == TT - 1))
            nc.vector.tensor_copy(yT8[:, h * HT + dt_, :], ps_y[:])

    # pipeline: scores(h+1) lands before av(h) so the last softmax hides
    emit_kv(0)
    emit_scores(0)
    for h in range(H):
        if h + 1 < H:
            emit_kv(h + 1)
            emit_scores(h + 1)
        emit_av(h)
    psBC.release()
    pBC.release()
    pQ.release()

    # ---------------- Stage D: output projection (fp8) + residual + LN2
    pD = tc.alloc_tile_pool(name="pD", bufs=2)
    psD = tc.alloc_tile_pool(name="psD", bufs=8, space="PSUM")
    # prefetch the first MLP1 weight chunks now: p_w1h predates the attention
    # pools, so these DMAs don't inherit LN2's address-alias waits
    for i in range(NSPLIT):
        nc.gpsimd.dma_start(w1hs[i][:], d["w1"][i])
    x_rds = []
    for qt in range(RT):
        x_rd = pD.tile([P, C], F32, name=f"x_rd{qt}", tag="x_rd", bufs=RT)
        eng = nc.sync if qt % 2 == 0 else nc.gpsimd
        eng.dma_start(x_rd[:, 0:1024], d["x"][qt * P:(qt + 1) * P, 0:1024])
        eng.dma_start(x_rd[:, 1024:C], d["x"][qt * P:(qt + 1) * P, 1024:C])
        x_rds.append(x_rd)

    # per-qt: proj MMs -> drain (+bo, +x) -> LN2 stats/xh -> transpose;
    # the w*xh+b apply fuses into the per-ki ACT slab ops below.
    for qt in range(RT):
        ps4 = [psD.tile([P, 512], F32, name=f"ps_o{fc}", tag="psD", bufs=8)
               for fc in range(4)]
        for fip in range(KP):
            for fc in range(4):
                nc.tensor.matmul(ps4[fc][:],
                                 yT8[:, 2 * fip:2 * fip + 2, qt * P:(qt + 1) * P],
                                 wo_t[:, fip, :, fc, :],
                                 start=(fip == 0), stop=(fip == KP - 1), perf_mode=DR)
        r_sb = pD.tile([P, C], BF16, name="r_sb", tag="r_sb", bufs=2)
        for fc in range(4):
            sl = slice(fc * 512, (fc + 1) * 512)
            nc.vector.scalar_tensor_tensor(r_sb[:, sl], ps4[fc][:], IWS,
                                           bo_bc[:, sl], OP.mult, OP.add)
            nc.vector.tensor_tensor(r_sb[:, sl], r_sb[:, sl], x_rds[qt][:, sl], OP.add)
        nc.sync.dma_start(r_d[qt * P:(qt + 1) * P, :], r_sb[:])
        stats = pD.tile([P, 4, 6], F32, name="ln2_stats", tag="ln2_stats", bufs=2)
        for sg in range(4):
            nc.vector.bn_stats(out=stats[:, sg, :], in_=r_sb[:, sg * 512:(sg + 1) * 512])
        mv = pD.tile([P, 2], F32, name="ln2_mv", tag="ln2_mv", bufs=2)
        nc.vector.bn_aggr(out=mv[:], in_=stats[:])
        std = pD.tile([P, 1], F32, name="ln2_std", tag="ln2_std", bufs=2)
        nc.scalar.activation(out=std[:], in_=mv[:, 1:2], func=ACT.Sqrt,
                             bias=eps_t[:], scale=1.0)
        rstd = pD.tile([P, 1], F32, name="ln2_rstd", tag="ln2_rstd", bufs=2)
        nc.vector.reciprocal(rstd[:], std[:])
        nmr = pD.tile([P, 1], F32, name="ln2_nmr", tag="ln2_nmr", bufs=2)
        nc.vector.tensor_scalar(nmr[:], mv[:, 0:1], rstd[:], -1.0, OP.mult, OP.mult)
        xh = pD.tile([P, C], BF16, name=f"ln2_xh{qt}", tag="ln2_xh", bufs=2)
        nc.scalar.activation(out=xh[:], in_=r_sb[:], func=ACT.Identity,
                             bias=nmr[:], scale=rstd[:])
        nc.sync.dma_start_transpose(xh2T[:, :, qt * P:(qt + 1) * P], xh[:])
    psD.release()
    pD.release()
    p_wo.release()

    # ---------------- Stage F: MLP up + gelu -> hT [128, FT, R] bf16
    p_hT = tc.alloc_tile_pool(name="p_hT", bufs=1)
    hT = p_hT.tile([P, FT, R], BF16, name="hT")
    pF = tc.alloc_tile_pool(name="pF", bufs=2)
    psF = tc.alloc_tile_pool(name="psF", bufs=4, space="PSUM")
    # The first fo-tiles run as row-halves: the [0:256] halves depend only on
    # LN2 of qt 0-1, so the PE bridges the LN2(q2/q3) tail instead of idling.
    w1cs, ps_hs = w1hs, []
    for rh in range(2):
        rs = slice(rh * 256, (rh + 1) * 256)
        for fp in range(NSPLIT):
            for fl in range(2):
                fo = 2 * fp + fl
                if rh == 0:
                    ps_h = psF.tile([P, R], F32, name="ps_h", tag="psF", bufs=8)
                    ps_hs.append(ps_h)
                ps_h = ps_hs[fo]
                for ki in range(CT):
                    nc.tensor.matmul(ps_h[:, rs], w1cs[fp][:, ki, fl, :],
                                     xh2T[:, ki, rs], start=(ki == 0),
                                     stop=(ki == CT - 1))
                if rh == 1:
                    nc.scalar.activation(out=hT[:, fo, :], in_=ps_h[:], func=ACT.Gelu,
                                         bias=b1_t[:, fo:fo + 1], scale=1.0)
    for fp in range(NSPLIT, FT // 2):
        w1c = pF.tile([P, CT, 2, P], BF16, name="w1c", tag="w1c", bufs=3)
        nc.sync.dma_start(w1c[:], d["w1"][fp])
        for fl in range(2):
            fo = 2 * fp + fl
            ps_h = psF.tile([P, R], F32, name="ps_h", tag="psF", bufs=8)
            for ki in range(CT):
                nc.tensor.matmul(ps_h[:], w1c[:, ki, fl, :], xh2T[:, ki, :],
                                 start=(ki == 0), stop=(ki == CT - 1))
            nc.scalar.activation(out=hT[:, fo, :], in_=ps_h[:], func=ACT.Gelu,
                                 bias=b1_t[:, fo:fo + 1], scale=1.0)
    psF.release()
    pF.release()

    # ---------------- Stage G: MLP down (w2 streamed, hT stationary) + residual
    pG = tc.alloc_tile_pool(name="pG", bufs=2)
    psG = tc.alloc_tile_pool(name="psG", bufs=8, space="PSUM")
    HOT = FT - 8                     # tail ho-block finished per-qt (staggered drain)
    for half in range(2):
        co0 = half * 1024
        ps8 = [psG.tile([P, 512], F32, name=f"ps_g{i}", tag="psG", bufs=8)
               for i in range(8)]
        # rb = r + b2, computed under the matmul stream -> 1-op drains
        rbs = []
        for qt in range(RT):
            rr = pG.tile([P, 1024], BF16, name="rr", tag="rr", bufs=RT)
            nc.sync.dma_start(rr[:], r_d[qt * P:(qt + 1) * P, co0:co0 + 1024])
            nc.vector.tensor_tensor(rr[:], rr[:], b2_bc[:, co0:co0 + 1024], OP.add)
            rbs.append(rr)
        for ho in range(HOT):
            w2b = pG.tile([P, 1024], BF16, name="w2b", tag="w2b", bufs=4)
            nc.sync.dma_start(w2b[:], d["w2"][half, ho])
            for qt in range(RT):
                for fc in range(2):
                    nc.tensor.matmul(ps8[qt * 2 + fc][:], hT[:, ho, qt * P:(qt + 1) * P],
                                     w2b[:, fc * 512:(fc + 1) * 512],
                                     start=(ho == 0), stop=False)
        w2t = pG.tile([P, 8, 1024], BF16, name="w2t", tag="w2t", bufs=2)
        for hl in range(8):
            nc.sync.dma_start(w2t[:, hl, :], d["w2"][half, HOT + hl])
        for qt in range(RT):
            for hl in range(8):
                for fc in range(2):
                    nc.tensor.matmul(ps8[qt * 2 + fc][:],
                                     hT[:, HOT + hl, qt * P:(qt + 1) * P],
                                     w2t[:, hl, fc * 512:(fc + 1) * 512],
                                     start=False, stop=(hl == 7))
            for fc in range(2):
                sl = slice(co0 + fc * 512, co0 + (fc + 1) * 512)
                o_t = pG.tile([P, 512], F32, name="o_t", tag="o_t", bufs=4)
                nc.vector.tensor_tensor(o_t[:], ps8[qt * 2 + fc][:],
                                        rbs[qt][:, fc * 512:(fc + 1) * 512], OP.add)
                eng = nc.sync if fc == 0 else nc.scalar
                eng.dma_start(d["out"][qt * P:(qt + 1) * P, sl], o_t[:])
    psG.release()
    pG.release()
    p_hT.release()
    p_w1h.release()
    p_xnT8.release()
    p_yT.release()
    p_xn2T.release()
    p_rd.release()
    consts.release()


def build_program():
    nc = bacc.Bacc("TRN2", target_bir_lowering=False, debug=False, num_devices=8)
    with tile.TileContext(nc) as tc:
        _body(tc)
    nc.compile()
    return nc


_prog = None


def _get_prog():
    global _prog
    if _prog is None:
        _prog = build_program()
    return _prog


def make_in_maps(x, mask, Wq, bq, Wk, bk, Wv, bv, Wo, bo,
                 ln1_w, ln1_b, ln2_w, ln2_b, W1, b1, W2, b2):
    bf = ml_dtypes.bfloat16
    f8 = ml_dtypes.float8_e4m3
    f32 = np.float32
    cc = np.ascontiguousarray

    def f(a):
        return np.asarray(a, dtype=f32)

    def w8(a):
        return (f(a) * WS).astype(f8)

    x, mask = np.asarray(x, dtype=f32), np.asarray(mask)
    # fold LayerNorm affine (w, b) into the downstream projections: the device
    # only computes xh = (x - mu) * rstd, and  xh @ (diag(w) W) + (b @ W + c)
    # == (xh * w + b) @ W + c  exactly.
    l1w, l1b = f(ln1_w), f(ln1_b)
    l2w, l2b = f(ln2_w), f(ln2_b)
    Wq_f, bq_f = l1w[:, None] * f(Wq), f(bq) + l1b @ f(Wq)
    Wk_f, bk_f = l1w[:, None] * f(Wk), f(bk) + l1b @ f(Wk)
    Wv_f, bv_f = l1w[:, None] * f(Wv), f(bv) + l1b @ f(Wv)
    W1_f, b1_f = l2w[:, None] * f(W1), f(b1) + l2b @ f(W1)
    wq_h = cc(w8(Wq_f).reshape(CT, P, CT, P).transpose(2, 1, 0, 3))
    wk_h = cc(w8(Wk_f).reshape(CT, P, CT, P).transpose(2, 1, 0, 3))
    wv_h = cc(w8(Wv_f).reshape(CT, P, H, DH).transpose(2, 1, 0, 3))
    wo_h = cc(w8(Wo).reshape(KP, 2, P, 4, 512).transpose(0, 2, 1, 3, 4))
    w1_h = cc(W1_f.astype(bf).reshape(CT, P, FT // 2, 2, P).transpose(2, 1, 0, 3, 4))
    w2_h = cc(f(W2).astype(bf).reshape(FT, P, 2, 1024).transpose(2, 0, 1, 3))
    shared = dict(
        wq=wq_h, wk=wk_h, wv=wv_h, wo=wo_h, w1=w1_h, w2=w2_h,
        bq=cc(bq_f.reshape(CT, P).T), bk=cc(bk_f.reshape(CT, P).T),
        b1=cc(b1_f.reshape(FT, P).T),
        bv=bv_f.astype(bf), bo=f(bo).astype(bf), b2=f(b2).astype(bf),
    )
    in_maps = []
    for c in range(8):
        b, hh = divmod(c, 2)
        xc = np.roll(x[b], -hh * R, axis=0)
        mk = np.roll(np.asarray(mask[b, hh * R:(hh + 1) * R, :], dtype=f32),
                     -hh * R, axis=1).astype(bf)
        in_maps.append({**shared, "x": cc(xc), "mask": cc(mk)})
    return in_maps


def kernel(**inputs):
    nc = _get_prog()
    in_maps = make_in_maps(**inputs)
    res = run_bass_kernel_spmd(nc, in_maps, core_ids=list(range(8)))
    out = np.empty((B, T, C), np.float32)
    for c in range(8):
        b, hh = divmod(c, 2)
        out[b, hh * R:(hh + 1) * R, :] = res.results[c]["out"]
    return out
